# revision 33
# baseline (speedup 1.0000x reference)
"""LogLinearAttention TRN2 kernel: 8-core data-parallel over (batch, token-half).

Each core handles 2048 tokens (4 chunks of 512) of one batch element:
  core i -> batch i//2, tokens (i%2)*2048 ... +2048.
Block-local attention makes (batch, chunk) units fully independent.

This problem is wire-bound: the axon tunnel moves ~40-50 MB/s, so the
dispatch cost is dominated by host<->device bytes, not FLOPs. The design
minimizes bytes on the wire:
  - x ships as 12-bit fixed point [feat, tok] per core: a uint8 high-byte
    plane + packed low nibbles + per-core (xmin, delta), 25 MB total vs
    64 MB fp32; reconstructed on-device with an exact integer op chain.
  - Weights ship *sharded*: each core uploads a distinct 1/8 column-slice of
    (WqkvT | WoutT) packed as one [1024, 512] fp16 tensor (8 MB total vs
    128 MB duplicated fp32); an on-device AllGather over NeuronLink
    reassembles the full weights in HBM, then they are loaded to SBUF once
    and stay resident for all chunks.
  - The output ships as int8 with a per-(row, tile) scale: each [128, 512]
    output tile is quantized by its per-partition max/127 (round-to-nearest
    in the ACT int8 cast), bounding the normalized-max error at ~1/254 plus
    fp16 compute noise (~8e-4); measured end-to-end ~4e-3 vs the 2e-2 gate.
  - No donated output buffers: under the bass2jax axon path the NEFF outputs
    are custom-call results, and this kernel writes every output element, so
    the zero-buffer upload run_bass_via_pjrt does is dropped entirely.
  - Input-independent tensors (positions, head selector) are device-resident
    constants reused across dispatches; the remaining small inputs pack into
    one tensor (each input tensor costs 8 per-shard upload RPCs).

Compute layout (matmuls out = lhsT.T @ rhs, fp16 operands, fp32 PSUM):
  - qkv projection emits qT,kT as [feat, tok] (weights stationary); v emitted
    as [tok, feat] via the transposed orientation (x stationary).
  - scores computed as S.T [key, q]; key gate lam applied per-partition as the
    activation scale of a fused exp (no max-subtraction: |gated scores| < 6,
    exp < 65504/fp16 max).
  - AV uses a ones-augmented V (65th column) so the softmax denominator comes
    out as row 64 of the same accumulation; numerators accumulate to ~1e4 so
    the attnT staging tile stays fp32.
  - normalization defers to a selector-matmul broadcast of 1/sum applied to
    attn_out.T, which is exactly the lhsT/rhs layout the output projection
    needs. Output is written transposed; host transposes back.
"""
import sys
sys.path.insert(0, "/opt/trn_rl_repo")
import numpy as np
import concourse.bass as bass
import concourse.mybir as mybir
from concourse.tile import TileContext

B, T, C = 4, 4096, 1024
H, NCH = 16, 8
DH = C // H          # 64
CS = T // NCH        # 512 chunk size
TPC = T // 2         # tokens per core (2048)
NCHC = TPC // CS     # chunks per core (4)
NCORES = 8
WQC = 3 * C // NCORES  # 384 wqkv cols per core
WOC = C // NCORES      # 128 wout cols per core
WPK = WQC + WOC        # 512 packed weight cols per core
F32R = mybir.dt.float32r
F32 = mybir.dt.float32
F16 = mybir.dt.float16
I8 = mybir.dt.int8
U8 = mybir.dt.uint8
U16 = mybir.dt.uint16


def split_multi_waits(nc):
    """This walrus build allows one sync wait per instruction; hoist extras
    into NoOps on the same engine queue."""
    for f in nc.m.functions:
        for blk in f.blocks:
            new_insts = []
            for inst in blk.instructions:
                si = inst.sync_info
                if si is not None and si.on_wait is not None and len(si.on_wait) > 1:
                    waits = list(si.on_wait)
                    for j, w in enumerate(waits[:-1]):
                        nop = mybir.InstNoOp(
                            name=f"{inst.name}-ws{j}", engine=inst.engine, ins=[], outs=[]
                        )
                        nop.sync_info = mybir.SyncInfo(on_wait=[w], on_update=[])
                        new_insts.append(nop)
                    inst.sync_info = mybir.SyncInfo(
                        on_wait=[waits[-1]], on_update=list(si.on_update or [])
                    )
                new_insts.append(inst)
            blk.instructions = new_insts


def build_program():
    nc = bass.Bass(target_bir_lowering=False, trn_type="TRN2", num_devices=NCORES)
    AF = mybir.ActivationFunctionType

    # Per-dispatch inputs: x, the weight shard, and one packed tensor with
    # the lam-MLP weights + output bias (everything else input-dependent).
    # The lam MLP stays f32r: fp16 operands in its odd-shaped matmuls
    # ([65,128] f16 lhsT) crash the PE on this silicon.
    # x ships as 12-bit fixed point: a high-byte plane + packed low nibbles
    # (pairs of adjacent tokens), with a per-core (xmin, delta) affine in
    # lamw32. Reconstructed on-device with exact integer ops (1.5 B/elem).
    xhi_d = nc.dram_tensor("xhi", [C, TPC], U8, kind="ExternalInput")
    xlo_d = nc.dram_tensor("xlo", [C, TPC // 2], U8, kind="ExternalInput")
    w_d = nc.dram_tensor("w16", [C, WPK], F16, kind="ExternalInput")
    # lamw32 layout (f32): [0:128] l1_lhsT(2x64) | [128:1168] l2_rhs(65x16)
    #   | [1168:2192] bout2d(128x8) | [2192:2320] delta*128 | [2320:2448] xmin*128
    lamw_d = nc.dram_tensor("lamw32", [1, 2448], F32R, kind="ExternalInput")
    # Input-independent constants, uploaded once and cached on device:
    # logpos rows: row0=ln(pos+1) (per-core positions), row1=ones.
    logpos_d = nc.dram_tensor("logpos16", [2, TPC], F16, kind="ExternalInput")
    lpos32_d = nc.dram_tensor("logpos32", [2, TPC], F32R, kind="ExternalInput")
    sel_d = nc.dram_tensor("sel", [16, C], F32R, kind="ExternalInput")
    outq_d = nc.dram_tensor("outq", [C, TPC], I8, kind="ExternalOutput")
    oscale_d = nc.dram_tensor("oscale", [128, 32], F32, kind="ExternalOutput")

    with TileContext(nc) as tc, nc.allow_low_precision(reason="fp16 operands; accumulation stays fp32"):
        with tc.tile_pool(name="dram", bufs=1, space="DRAM") as dram, \
             tc.tile_pool(name="const", bufs=1) as cst, \
             tc.tile_pool(name="wq", bufs=1) as wqp, \
             tc.tile_pool(name="wv", bufs=1) as wvp, \
             tc.tile_pool(name="wo", bufs=1) as wop, \
             tc.tile_pool(name="ps", bufs=8, space="PSUM") as ps:

            # ---- weight AllGather: 1/8 slice per core -> full weights in HBM
            w_bounce = dram.tile([C, WPK], F16, name="w_bounce", tag="wb")
            w_gath = dram.tile([NCORES * C, WPK], F16, name="w_gath", tag="wg")
            nc.gpsimd.dma_start(w_bounce[:], w_d.ap())
            nc.gpsimd.collective_compute(
                "AllGather",
                mybir.AluOpType.bypass,
                replica_groups=[list(range(NCORES))],
                ins=[w_bounce.opt()],
                outs=[w_gath.opt()],
            )
            # rank-major gathered layout: row r*C + i, col j
            #   j < 384: wqkvT[i, r*384 + j];  j >= 384: woutT[i, r*128 + (j-384)]

            # ---- resident weight tiles (loaded once, used all chunks) ----
            # q/k projection lhsT tiles: wqk[ct][p, kt*128+m] = wqkvT[kt*128+p, ct*128+m]
            wqk_sb = []
            for ct in range(16):
                rk, lc = ct // 3, ct % 3
                wt = wqp.tile([128, C], F16, name=f"wqk{ct}", tag=f"wqk{ct}")
                nc.gpsimd.dma_start(
                    out=wt.rearrange("p (kt m) -> p kt m", m=128),
                    in_=w_gath[rk * C:(rk + 1) * C, lc * 128:(lc + 1) * 128]
                        .rearrange("(kt p) m -> p kt m", p=128))
                wqk_sb.append(wt)
            # v projection rhs tiles: wv[g*8+kt][p, j*128+m] = wqkvT[kt*128+p, 2048+g*512+j*128+m]
            wv_sb = []
            for g in range(2):
                for kt in range(8):
                    wv = wvp.tile([128, 512], F16, name=f"wv{g}_{kt}", tag=f"wv{g}_{kt}")
                    for j in range(4):
                        vt = 16 + g * 4 + j
                        rk, lc = vt // 3, vt % 3
                        nc.gpsimd.dma_start(
                            out=wv[:, j * 128:(j + 1) * 128],
                            in_=w_gath[rk * C + kt * 128:rk * C + (kt + 1) * 128,
                                       lc * 128:(lc + 1) * 128])
                    wv_sb.append(wv)
            # out projection lhsT tiles: wout[kt][p, of*128+m] = woutT[kt*128+p, of*128+m]
            wout_sb = []
            for kt in range(8):
                w = wop.tile([128, C], F16, name=f"wout{kt}", tag=f"wout{kt}")
                for of in range(8):
                    nc.gpsimd.dma_start(
                        out=w[:, of * 128:(of + 1) * 128],
                        in_=w_gath[of * C + kt * 128:of * C + (kt + 1) * 128,
                                   WQC:WPK])
                wout_sb.append(w)

            bout_sb = cst.tile([128, 8], F32R, tag="bout")
            nc.sync.dma_start(
                out=bout_sb[:],
                in_=lamw_d.ap()[0:1, 1168:2192].rearrange("r (p m) -> (r p) m", p=128))
            sel_sb = cst.tile([16, C], F32R, tag="sel")
            nc.sync.dma_start(out=sel_sb[:], in_=sel_d.ap())
            l1_sb = cst.tile([2, 64], F32R, tag="l1")
            nc.sync.dma_start(
                out=l1_sb[:],
                in_=lamw_d.ap()[0:1, 0:128].rearrange("r (p m) -> (r p) m", p=2))
            l2_sb = cst.tile([65, 16], F32R, tag="l2")
            nc.sync.dma_start(
                out=l2_sb[:],
                in_=lamw_d.ap()[0:1, 128:1168].rearrange("r (p m) -> (r p) m", p=65))
            # f32 tiles from the f32r-declared pack: byte-identical, but only
            # gpsimd DMAs may reinterpret dtypes.
            xdl_sb = cst.tile([128, 1], F32, tag="xdl")
            nc.gpsimd.dma_start(
                out=xdl_sb[:],
                in_=lamw_d.ap()[0:1, 2192:2320].rearrange("r (p m) -> (r p) m", p=128))
            xmn_sb = cst.tile([128, 1], F32, tag="xmn")
            nc.gpsimd.dma_start(
                out=xmn_sb[:],
                in_=lamw_d.ap()[0:1, 2320:2448].rearrange("r (p m) -> (r p) m", p=128))
            oscale_sb = cst.tile([128, 32], F32, tag="oscale")

            # ---- lambda gate MLP: lam_sb[key_part, tt*16+h] ----
            lam_sb = cst.tile([128, 16 * 16], F32, tag="lam")
            with tc.tile_pool(name="pre", bufs=1) as pre:
                h1_sb = pre.tile([65, TPC], F32R, tag="h1")
                lp = pre.tile([2, TPC], F32R, tag="lp")
                nc.sync.dma_start(out=lp[:], in_=lpos32_d.ap())
                nc.sync.dma_start(out=h1_sb[64:65, :], in_=lpos32_d.ap()[1:2, :])
                for j in range(4):
                    ph = ps.tile([64, 512], F32, name=f"ph{j}", tag="b512")
                    nc.tensor.matmul(ph[:], l1_sb[:], lp[:, j * 512:(j + 1) * 512], start=True, stop=True)
                    nc.scalar.activation(h1_sb[0:64, j * 512:(j + 1) * 512], ph[:], AF.Relu)
                for tt in range(16):
                    pl = ps.tile([128, 16], F32, name=f"pl{tt}", tag="b512")
                    nc.tensor.matmul(pl[:], h1_sb[:, tt * 128:(tt + 1) * 128], l2_sb[:],
                                     start=True, stop=True)
                    nc.scalar.activation(lam_sb[:, tt * 16:(tt + 1) * 16], pl[:], AF.Sigmoid)

            # ---- main loop over chunks ----
            ctx2 = [tc.tile_pool(name="xs", bufs=12), tc.tile_pool(name="qk", bufs=16),
                    tc.tile_pool(name="vv", bufs=4), tc.tile_pool(name="pt", bufs=5),
                    tc.tile_pool(name="at", bufs=8), tc.tile_pool(name="atn", bufs=8),
                    tc.tile_pool(name="ob", bufs=3), tc.tile_pool(name="qb", bufs=3),
                    tc.tile_pool(name="xu", bufs=2)]
            xs = ctx2[0].__enter__(); qk = ctx2[1].__enter__(); vv = ctx2[2].__enter__()
            ptp = ctx2[3].__enter__(); at = ctx2[4].__enter__(); atn = ctx2[5].__enter__()
            ob = ctx2[6].__enter__(); qb = ctx2[7].__enter__(); xu = ctx2[8].__enter__()
            ALU = mybir.AluOpType
            for c in range(NCHC):
                # x tiles for this chunk: rows = features, cols = tokens.
                # Unpack 12-bit fixed point -> f16 (exact integer chain,
                # validated bit-exact on-device in isolation).
                x_sb = []
                for kt in range(8):
                    hi = xu.tile([128, 512], U8, name=f"xh_c{c}_k{kt}", tag="xh")
                    nc.sync.dma_start(
                        out=hi[:],
                        in_=xhi_d.ap()[kt * 128:(kt + 1) * 128, c * 512:(c + 1) * 512])
                    lo = xu.tile([128, 256], U8, name=f"xl_c{c}_k{kt}", tag="xl")
                    nc.sync.dma_start(
                        out=lo[:],
                        in_=xlo_d.ap()[kt * 128:(kt + 1) * 128, c * 256:(c + 1) * 256])
                    hi16 = xu.tile([128, 512], U16, name=f"xh16_c{c}_k{kt}", tag="xh16")
                    nc.vector.tensor_copy(hi16[:], hi[:])
                    hi16s = xu.tile([128, 512], U16, name=f"xh16s_c{c}_k{kt}", tag="xh16s")
                    nc.vector.tensor_scalar(hi16s[:], hi16[:], 4, None, ALU.logical_shift_left)
                    lof = xu.tile([128, 512], U8, name=f"xlf_c{c}_k{kt}", tag="xlf")
                    lof3 = lof.rearrange("p (e two) -> p e two", two=2)
                    lo3 = lo[:].rearrange("p (e o) -> p e o", o=1)
                    nc.vector.tensor_scalar(lof3[:, :, 0:1], lo3, 15, None, ALU.bitwise_and)
                    nc.vector.tensor_scalar(lof3[:, :, 1:2], lo3, 4, None, ALU.logical_shift_right)
                    lo16 = xu.tile([128, 512], U16, name=f"xl16_c{c}_k{kt}", tag="xl16")
                    nc.vector.tensor_copy(lo16[:], lof[:])
                    q16 = xu.tile([128, 512], U16, name=f"xq16_c{c}_k{kt}", tag="xq16")
                    nc.vector.tensor_tensor(q16[:], hi16s[:], lo16[:], ALU.bitwise_or)
                    qf = xu.tile([128, 512], F32, name=f"xqf_c{c}_k{kt}", tag="xqf")
                    nc.vector.tensor_copy(qf[:], q16[:])
                    xt = xs.tile([128, 512], F16, name=f"x_c{c}_k{kt}", tag="x")
                    nc.vector.tensor_scalar(xt[:], qf[:], xdl_sb[:, 0:1], xmn_sb[:, 0:1],
                                            ALU.mult, ALU.add)
                    x_sb.append(xt)

                # q,k projection: out.T tiles [feat 128, tok 512], feats 0..2047
                qkT = []
                for ct in range(16):
                    pq = ps.tile([128, 512], F32, name=f"pq_c{c}_{ct}", tag="b512")
                    for kt in range(8):
                        nc.tensor.matmul(pq[:], wqk_sb[ct][:, kt * 128:(kt + 1) * 128],
                                         x_sb[kt][:], start=(kt == 0), stop=(kt == 7))
                    qt = qk.tile([128, 512], F16, name=f"qkT_c{c}_{ct}", tag="qkT")
                    nc.scalar.copy(qt[:], pq[:])
                    qkT.append(qt)

                # v projection (x stationary): v_sb[tt] = [tok 128, 16*(64+1)]
                v_sb = []
                for tt in range(4):
                    vt = vv.tile([128, 16 * 65], F16, name=f"v_c{c}_{tt}", tag="v")
                    v3 = vt.rearrange("p (h e) -> p h e", e=65)
                    nc.sync.dma_start(
                        out=v3[:, :, 64:65],
                        in_=logpos_d.ap()[1:2, 0:2048].rearrange("r (p e o) -> (r p) e o", p=128, o=1))
                    v_sb.append(vt)
                for g in range(2):
                    for tt in range(4):
                        pv = ps.tile([128, 512], F32, name=f"pv_c{c}_{g}_{tt}", tag="b512")
                        for kt in range(8):
                            nc.tensor.matmul(pv[:], x_sb[kt][:, tt * 128:(tt + 1) * 128],
                                             wv_sb[g * 8 + kt][:],
                                             start=(kt == 0), stop=(kt == 7))
                        dst = v_sb[tt].rearrange("p (h e) -> p h e", e=65)[:, g * 8:(g + 1) * 8, 0:64]
                        src = pv.rearrange("p (h e) -> p h e", e=64)
                        nc.scalar.copy(dst, src)

                # attention per head
                sums_sb = at.tile([16, 512], F32, name=f"sums_c{c}", tag="sums")
                attnT = []
                for hp in range(8):
                    a_t = at.tile([128, 512], F32, name=f"attnT_c{c}_{hp}", tag="attnT")
                    attnT.append(a_t)
                for h in range(16):
                    hp, r0 = h // 2, (h % 2) * 64
                    kt_tile = qkT[8 + h // 2]
                    qt_tile = qkT[h // 2]
                    p_ts = []
                    for kk in range(4):
                        pscr = ps.tile([128, 512], F32, name=f"ps_c{c}_h{h}_{kk}", tag="b512")
                        nc.tensor.matmul(pscr[:],
                                         kt_tile[r0:r0 + 64, kk * 128:(kk + 1) * 128],
                                         qt_tile[r0:r0 + 64, :], start=True, stop=True)
                        p_t = ptp.tile([128, 512], F16, name=f"p_c{c}_h{h}_{kk}", tag="p")
                        tt = c * 4 + kk
                        nc.scalar.activation(p_t[:], pscr[:], AF.Exp,
                                             scale=lam_sb[:, tt * 16 + h:tt * 16 + h + 1])
                        p_ts.append(p_t)
                    pav = ps.tile([128, 512], F32, name=f"pav_c{c}_h{h}", tag="b512")
                    for kk in range(4):
                        nc.tensor.matmul(pav[0:65, :],
                                         v_sb[kk][:, h * 65:(h + 1) * 65],
                                         p_ts[kk][:], start=(kk == 0), stop=(kk == 3))
                    nc.scalar.copy(attnT[hp][r0:r0 + 64, :], pav[0:64, :])
                    srow = at.tile([1, 512], F32, name=f"srow_c{c}_h{h}", tag="srow", bufs=2)
                    nc.scalar.copy(srow[:], pav[64:65, :])
                    nc.sync.dma_start(out=sums_sb[h:h + 1, :], in_=srow[:])

                # normalization via selector broadcast of 1/sums
                inv_sb = at.tile([16, 512], F32R, name=f"inv_c{c}", tag="inv")
                nc.vector.reciprocal(inv_sb[:], sums_sb[:])
                attnTn = []
                for hp in range(8):
                    pg = ps.tile([128, 512], F32, name=f"pg_c{c}_{hp}", tag="b512")
                    nc.tensor.matmul(pg[:], sel_sb[:, hp * 128:(hp + 1) * 128], inv_sb[:],
                                     start=True, stop=True)
                    an = atn.tile([128, 512], F16, name=f"attnTn_c{c}_{hp}", tag="an")
                    nc.vector.tensor_mul(an[:], attnT[hp][:], pg[:])
                    attnTn.append(an)

                # output projection + int8 quantization with per-row scales
                for of in range(8):
                    po = ps.tile([128, 512], F32, name=f"po_c{c}_{of}", tag="b512")
                    for kt in range(8):
                        nc.tensor.matmul(po[:], wout_sb[kt][:, of * 128:(of + 1) * 128],
                                         attnTn[kt][:], start=(kt == 0), stop=(kt == 7))
                    o_sb = ob.tile([128, 512], F32, name=f"o_c{c}_{of}", tag="o")
                    nc.scalar.activation(o_sb[:], po[:], AF.Identity,
                                         bias=bout_sb[:, of:of + 1])
                    mx = ob.tile([128, 1], F32, name=f"mx_c{c}_{of}", tag="mx", bufs=3)
                    nc.vector.tensor_reduce(mx[:], o_sb[:], mybir.AxisListType.X,
                                            mybir.AluOpType.max, apply_absolute_value=True)
                    inv127 = ob.tile([128, 1], F32, name=f"i127_c{c}_{of}", tag="i127", bufs=3)
                    nc.vector.reciprocal(inv127[:], mx[:])
                    i127s = ob.tile([128, 1], F32, name=f"i127s_c{c}_{of}", tag="i127s", bufs=3)
                    nc.scalar.activation(i127s[:], inv127[:], AF.Identity, scale=127.0)
                    nc.scalar.activation(oscale_sb[:, (of * 4 + c):(of * 4 + c) + 1],
                                         mx[:], AF.Identity, scale=1.0 / 127.0)
                    qt8 = qb.tile([128, 512], I8, name=f"q_c{c}_{of}", tag="q8")
                    nc.scalar.activation(qt8[:], o_sb[:], AF.Identity, scale=i127s[:])
                    nc.sync.dma_start(
                        out=outq_d.ap()[of * 128:(of + 1) * 128, c * 512:(c + 1) * 512],
                        in_=qt8[:])
            nc.sync.dma_start(out=oscale_d.ap(), in_=oscale_sb[:])
            for cm in reversed(ctx2):
                cm.__exit__(None, None, None)

    split_multi_waits(nc)
    return nc


_cache = {}


def _get_exec():
    """Build the program + cached jitted SPMD dispatcher (mirrors
    bass2jax.run_bass_via_pjrt minus per-call retrace and minus the
    donated zero output buffers)."""
    if "exec" in _cache:
        return _cache["exec"]
    import jax
    from jax.sharding import Mesh, PartitionSpec, NamedSharding
    from jax.experimental.shard_map import shard_map
    from concourse.bass2jax import (
        _bass_exec_p, install_neuronx_cc_hook, partition_id_tensor)

    nc = build_program()
    install_neuronx_cc_hook()

    partition_name = nc.partition_id_tensor.name if nc.partition_id_tensor else None
    in_names, out_names, out_avals = [], [], []
    for alloc in nc.m.functions[0].allocations:
        if not isinstance(alloc, mybir.MemoryLocationSet):
            continue
        name = alloc.memorylocations[0].name
        if alloc.kind == "ExternalInput":
            if name != partition_name:
                in_names.append(name)
        elif alloc.kind == "ExternalOutput":
            out_names.append(name)
            out_avals.append(jax.core.ShapedArray(
                tuple(alloc.tensor_shape), mybir.dt.np(alloc.dtype)))
    n_params = len(in_names)
    n_outs = len(out_avals)
    # NEFF output tensors are custom-call *results* (out_rename overrides
    # the input binding in neuronx_cc_hook), and this kernel writes every
    # output element — so no donated zero buffers are needed at all.
    in_names_full = in_names + ([partition_name] if partition_name else [])

    def _body(*args):
        operands = list(args)
        if partition_name is not None:
            operands.append(partition_id_tensor())
        return tuple(_bass_exec_p.bind(
            *operands, out_avals=tuple(out_avals), in_names=tuple(in_names_full),
            out_names=tuple(out_names), lowering_input_output_aliases=(),
            sim_require_finite=True, sim_require_nnan=True, nc=nc))

    devices = jax.devices()[:NCORES]
    mesh = Mesh(np.asarray(devices), ("core",))
    sh = NamedSharding(mesh, PartitionSpec("core"))
    in_specs = (PartitionSpec("core"),) * n_params
    out_specs = (PartitionSpec("core"),) * n_outs
    sharded = jax.jit(
        shard_map(_body, mesh=mesh, in_specs=in_specs, out_specs=out_specs,
                  check_rep=False),
        keep_unused=True)

    # Input-independent constants: upload once, reuse every dispatch.
    logpos32 = np.empty((NCORES * 2, TPC), np.float32)
    for core in range(NCORES):
        pos = (core % 2) * TPC + np.arange(TPC, dtype=np.float32)
        logpos32[core * 2] = np.log(pos + 1.0)
        logpos32[core * 2 + 1] = 1.0
    sel = (np.arange(C)[None, :] // DH == np.arange(H)[:, None]).astype(np.float32)
    sel_all = np.broadcast_to(sel, (NCORES, H, C)).reshape(NCORES * H, C)
    consts = {
        "logpos16": jax.device_put(logpos32.astype(np.float16), sh),
        "logpos32": jax.device_put(logpos32, sh),
        "sel": jax.device_put(np.ascontiguousarray(sel_all), sh),
    }
    jax.block_until_ready(list(consts.values()))

    ex = dict(nc=nc, sharded=sharded, in_names=in_names, out_names=out_names,
              consts=consts, devices=devices, sh=sh)
    _cache["exec"] = ex
    return ex


def dispatch(in_maps):
    """The timed region: host numpy in_maps -> per-core fp32 outT arrays.
    Uploads fp16 x + sharded fp16 weights, executes (on-device weight
    AllGather + attention), downloads int8 output + scales (per-shard in
    parallel threads — the tunnel fetch path is ~1.6x faster that way),
    dequantizes per core as each shard arrives."""
    from concurrent.futures import ThreadPoolExecutor
    import jax
    ex = _get_exec()
    consts = ex["consts"]
    sh = ex["sh"]
    devices = ex["devices"]
    args = []
    for name in ex["in_names"]:
        if name in consts:
            args.append(consts[name])
            continue
        # upload per-core shards directly (skips a host-side concat copy)
        shards = jax.device_put([m[name] for m in in_maps], devices)
        per = in_maps[0][name].shape[0]
        glob = jax.make_array_from_single_device_arrays(
            (NCORES * per, *in_maps[0][name].shape[1:]), sh, shards)
        args.append(glob)
    out_arrs = ex["sharded"](*args)
    q_shards = sorted(out_arrs[0].addressable_shards, key=lambda s: s.index[0].start)
    s_shards = sorted(out_arrs[1].addressable_shards, key=lambda s: s.index[0].start)

    with ThreadPoolExecutor(NCORES + 2) as pool:
        # scales are tiny; fetch them concurrently with the big int8 shards
        s_futs = [pool.submit(lambda shd: np.asarray(shd.data), shd) for shd in s_shards]

        def fetch_decode(core):
            q = np.asarray(q_shards[core].data).reshape(8, 128, 4, 512)
            s = s_futs[core].result().reshape(128, 8, 4).transpose(1, 0, 2)
            o = np.empty((8, 128, 4, 512), np.float32)
            np.multiply(q, s[..., None], out=o)
            return o.reshape(C, TPC)

        outs = list(pool.map(fetch_decode, range(NCORES)))
    return outs


def kernel(x, Wqkv, Wout, bout, Wl1, bl1, Wl2, bl2):
    scale = DH ** -0.5
    wqkvT = np.ascontiguousarray(Wqkv.T, dtype=np.float32)
    wqkvT[:, :C] *= scale  # fold attention scale into q projection (1/8: exact in fp16)
    woutT = Wout.T.astype(np.float32)
    lamw32 = np.empty((1, 2448), np.float32)
    lamw32[0, 0:128] = np.stack([Wl1[:, 0], bl1]).astype(np.float32).ravel()
    lamw32[0, 128:1168] = np.concatenate(
        [np.asarray(Wl2).T, np.asarray(bl2)[None, :]], 0).astype(np.float32).ravel()
    lamw32[0, 1168:2192] = np.asarray(bout).reshape(8, 128).T.astype(np.float32).ravel()

    xf = np.asarray(x, dtype=np.float32)
    wqkvT16 = wqkvT.astype(np.float16)
    woutT16 = woutT.astype(np.float16)

    in_maps = []
    for core in range(NCORES):
        b, half = core // 2, core % 2
        xT = xf[b, half * TPC:(half + 1) * TPC, :].T
        xmin = float(xT.min())
        delta = max((float(xT.max()) - xmin) / 4095.0, 1e-30)
        q = np.clip(np.round((xT - xmin) * (1.0 / delta)), 0, 4095).astype(np.uint16)
        xhi = (q >> 4).astype(np.uint8)
        lo4 = (q & 15).astype(np.uint8)
        xlo = np.ascontiguousarray(lo4[:, 0::2] | (lo4[:, 1::2] << 4))
        w16 = np.empty((C, WPK), np.float16)
        w16[:, :WQC] = wqkvT16[:, core * WQC:(core + 1) * WQC]
        w16[:, WQC:] = woutT16[:, core * WOC:(core + 1) * WOC]
        lamw = lamw32.copy()
        lamw[0, 2192:2320] = delta
        lamw[0, 2320:2448] = xmin
        in_maps.append(dict(xhi=np.ascontiguousarray(xhi), xlo=xlo,
                            w16=w16, lamw32=lamw))

    global _last_in_maps
    _last_in_maps = in_maps
    res = dispatch(in_maps)

    out = np.empty((B, T, C), np.float32)
    for core in range(NCORES):
        b, half = core // 2, core % 2
        out[b, half * TPC:(half + 1) * TPC, :] = res[core].T
    return out


# revision 37
# speedup vs baseline: 1.1297x; 1.1297x over previous
"""LogLinearAttention TRN2 kernel: 8-core data-parallel over (batch, token-half).

Each core handles 2048 tokens (4 chunks of 512) of one batch element:
  core i -> batch i//2, tokens (i%2)*2048 ... +2048.
Block-local attention makes (batch, chunk) units fully independent.

This problem is wire-bound: the axon tunnel moves ~40-50 MB/s, so the
dispatch cost is dominated by host<->device bytes, not FLOPs. The design
minimizes bytes on the wire:
  - x ships as 10-bit fixed point [feat, tok] per core: a uint8 high-byte
    plane + packed low 2-bit plane + per-core (xmin, delta), 21 MB total vs
    64 MB fp32; reconstructed on-device with an exact integer op chain.
  - Weights ship *sharded*: each core uploads a distinct 1/8 column-slice of
    (WqkvT | WoutT) packed as one [1024, 512] fp16 tensor (8 MB total vs
    128 MB duplicated fp32); an on-device AllGather over NeuronLink
    reassembles the full weights in HBM, then they are loaded to SBUF once
    and stay resident for all chunks.
  - The output ships as int8 with a per-(row, tile) scale: each [128, 512]
    output tile is quantized by its per-partition max/127 (round-to-nearest
    in the ACT int8 cast), bounding the normalized-max error at ~1/254 plus
    fp16 compute noise (~8e-4); measured end-to-end ~4e-3 vs the 2e-2 gate.
  - No donated output buffers: under the bass2jax axon path the NEFF outputs
    are custom-call results, and this kernel writes every output element, so
    the zero-buffer upload run_bass_via_pjrt does is dropped entirely.
  - Input-independent tensors (positions, head selector) are device-resident
    constants reused across dispatches; the remaining small inputs pack into
    one tensor (each input tensor costs 8 per-shard upload RPCs).

Compute layout (matmuls out = lhsT.T @ rhs, fp16 operands, fp32 PSUM):
  - qkv projection emits qT,kT as [feat, tok] (weights stationary); v emitted
    as [tok, feat] via the transposed orientation (x stationary).
  - scores computed as S.T [key, q]; key gate lam applied per-partition as the
    activation scale of a fused exp (no max-subtraction: |gated scores| < 6,
    exp < 65504/fp16 max).
  - AV uses a ones-augmented V (65th column) so the softmax denominator comes
    out as row 64 of the same accumulation; numerators accumulate to ~1e4 so
    the attnT staging tile stays fp32.
  - normalization defers to a selector-matmul broadcast of 1/sum applied to
    attn_out.T, which is exactly the lhsT/rhs layout the output projection
    needs. Output is written transposed; host transposes back.
"""
import sys
sys.path.insert(0, "/opt/trn_rl_repo")
import numpy as np
import concourse.bass as bass
import concourse.mybir as mybir
from concourse.tile import TileContext

B, T, C = 4, 4096, 1024
H, NCH = 16, 8
DH = C // H          # 64
CS = T // NCH        # 512 chunk size
TPC = T // 2         # tokens per core (2048)
NCHC = TPC // CS     # chunks per core (4)
NCORES = 8
WQC = 3 * C // NCORES  # 384 wqkv cols per core
WOC = C // NCORES      # 128 wout cols per core
WPK = WQC + WOC        # 512 packed weight cols per core
F32R = mybir.dt.float32r
F32 = mybir.dt.float32
F16 = mybir.dt.float16
I8 = mybir.dt.int8
U8 = mybir.dt.uint8
U16 = mybir.dt.uint16


def split_multi_waits(nc):
    """This walrus build allows one sync wait per instruction; hoist extras
    into NoOps on the same engine queue."""
    for f in nc.m.functions:
        for blk in f.blocks:
            new_insts = []
            for inst in blk.instructions:
                si = inst.sync_info
                if si is not None and si.on_wait is not None and len(si.on_wait) > 1:
                    waits = list(si.on_wait)
                    for j, w in enumerate(waits[:-1]):
                        nop = mybir.InstNoOp(
                            name=f"{inst.name}-ws{j}", engine=inst.engine, ins=[], outs=[]
                        )
                        nop.sync_info = mybir.SyncInfo(on_wait=[w], on_update=[])
                        new_insts.append(nop)
                    inst.sync_info = mybir.SyncInfo(
                        on_wait=[waits[-1]], on_update=list(si.on_update or [])
                    )
                new_insts.append(inst)
            blk.instructions = new_insts


def build_program():
    nc = bass.Bass(target_bir_lowering=False, trn_type="TRN2", num_devices=NCORES)
    AF = mybir.ActivationFunctionType

    # Per-dispatch inputs: x, the weight shard, and one packed tensor with
    # the lam-MLP weights + output bias (everything else input-dependent).
    # The lam MLP stays f32r: fp16 operands in its odd-shaped matmuls
    # ([65,128] f16 lhsT) crash the PE on this silicon.
    # x ships as 10-bit fixed point: a high-byte plane + packed low 2-bit
    # plane (4 adjacent tokens per byte), with a per-core (xmin, delta)
    # affine in lamw32. Reconstructed on-device with exact integer ops
    # (1.25 B/elem; measured end-to-end ~8.5e-3 vs the 2e-2 gate).
    xhi_d = nc.dram_tensor("xhi", [C, TPC], U8, kind="ExternalInput")
    xlo_d = nc.dram_tensor("xlo", [C, TPC // 4], U8, kind="ExternalInput")
    w_d = nc.dram_tensor("w16", [C, WPK], F16, kind="ExternalInput")
    # lamw32 layout (f32): [0:128] l1_lhsT(2x64) | [128:1168] l2_rhs(65x16)
    #   | [1168:2192] bout2d(128x8) | [2192:2320] delta*128 | [2320:2448] xmin*128
    lamw_d = nc.dram_tensor("lamw32", [1, 2448], F32R, kind="ExternalInput")
    # Input-independent constants, uploaded once and cached on device:
    # logpos rows: row0=ln(pos+1) (per-core positions), row1=ones.
    logpos_d = nc.dram_tensor("logpos16", [2, TPC], F16, kind="ExternalInput")
    lpos32_d = nc.dram_tensor("logpos32", [2, TPC], F32R, kind="ExternalInput")
    sel_d = nc.dram_tensor("sel", [16, C], F32R, kind="ExternalInput")
    outq_d = nc.dram_tensor("outq", [C, TPC], I8, kind="ExternalOutput")
    oscale_d = nc.dram_tensor("oscale", [128, 32], F32, kind="ExternalOutput")

    with TileContext(nc) as tc, nc.allow_low_precision(reason="fp16 operands; accumulation stays fp32"):
        with tc.tile_pool(name="dram", bufs=1, space="DRAM") as dram, \
             tc.tile_pool(name="const", bufs=1) as cst, \
             tc.tile_pool(name="wq", bufs=1) as wqp, \
             tc.tile_pool(name="wv", bufs=1) as wvp, \
             tc.tile_pool(name="wo", bufs=1) as wop, \
             tc.tile_pool(name="ps", bufs=8, space="PSUM") as ps:

            # ---- weight AllGather: 1/8 slice per core -> full weights in HBM
            w_bounce = dram.tile([C, WPK], F16, name="w_bounce", tag="wb")
            w_gath = dram.tile([NCORES * C, WPK], F16, name="w_gath", tag="wg")
            nc.gpsimd.dma_start(w_bounce[:], w_d.ap())
            nc.gpsimd.collective_compute(
                "AllGather",
                mybir.AluOpType.bypass,
                replica_groups=[list(range(NCORES))],
                ins=[w_bounce.opt()],
                outs=[w_gath.opt()],
            )
            # rank-major gathered layout: row r*C + i, col j
            #   j < 384: wqkvT[i, r*384 + j];  j >= 384: woutT[i, r*128 + (j-384)]

            # ---- resident weight tiles (loaded once, used all chunks) ----
            # q/k projection lhsT tiles: wqk[ct][p, kt*128+m] = wqkvT[kt*128+p, ct*128+m]
            wqk_sb = []
            for ct in range(16):
                rk, lc = ct // 3, ct % 3
                wt = wqp.tile([128, C], F16, name=f"wqk{ct}", tag=f"wqk{ct}")
                nc.gpsimd.dma_start(
                    out=wt.rearrange("p (kt m) -> p kt m", m=128),
                    in_=w_gath[rk * C:(rk + 1) * C, lc * 128:(lc + 1) * 128]
                        .rearrange("(kt p) m -> p kt m", p=128))
                wqk_sb.append(wt)
            # v projection rhs tiles: wv[g*8+kt][p, j*128+m] = wqkvT[kt*128+p, 2048+g*512+j*128+m]
            wv_sb = []
            for g in range(2):
                for kt in range(8):
                    wv = wvp.tile([128, 512], F16, name=f"wv{g}_{kt}", tag=f"wv{g}_{kt}")
                    for j in range(4):
                        vt = 16 + g * 4 + j
                        rk, lc = vt // 3, vt % 3
                        nc.gpsimd.dma_start(
                            out=wv[:, j * 128:(j + 1) * 128],
                            in_=w_gath[rk * C + kt * 128:rk * C + (kt + 1) * 128,
                                       lc * 128:(lc + 1) * 128])
                    wv_sb.append(wv)
            # out projection lhsT tiles: wout[kt][p, of*128+m] = woutT[kt*128+p, of*128+m]
            wout_sb = []
            for kt in range(8):
                w = wop.tile([128, C], F16, name=f"wout{kt}", tag=f"wout{kt}")
                for of in range(8):
                    nc.gpsimd.dma_start(
                        out=w[:, of * 128:(of + 1) * 128],
                        in_=w_gath[of * C + kt * 128:of * C + (kt + 1) * 128,
                                   WQC:WPK])
                wout_sb.append(w)

            bout_sb = cst.tile([128, 8], F32R, tag="bout")
            nc.sync.dma_start(
                out=bout_sb[:],
                in_=lamw_d.ap()[0:1, 1168:2192].rearrange("r (p m) -> (r p) m", p=128))
            sel_sb = cst.tile([16, C], F32R, tag="sel")
            nc.sync.dma_start(out=sel_sb[:], in_=sel_d.ap())
            l1_sb = cst.tile([2, 64], F32R, tag="l1")
            nc.sync.dma_start(
                out=l1_sb[:],
                in_=lamw_d.ap()[0:1, 0:128].rearrange("r (p m) -> (r p) m", p=2))
            l2_sb = cst.tile([65, 16], F32R, tag="l2")
            nc.sync.dma_start(
                out=l2_sb[:],
                in_=lamw_d.ap()[0:1, 128:1168].rearrange("r (p m) -> (r p) m", p=65))
            # f32 tiles from the f32r-declared pack: byte-identical, but only
            # gpsimd DMAs may reinterpret dtypes.
            xdl_sb = cst.tile([128, 1], F32, tag="xdl")
            nc.gpsimd.dma_start(
                out=xdl_sb[:],
                in_=lamw_d.ap()[0:1, 2192:2320].rearrange("r (p m) -> (r p) m", p=128))
            xmn_sb = cst.tile([128, 1], F32, tag="xmn")
            nc.gpsimd.dma_start(
                out=xmn_sb[:],
                in_=lamw_d.ap()[0:1, 2320:2448].rearrange("r (p m) -> (r p) m", p=128))
            oscale_sb = cst.tile([128, 32], F32, tag="oscale")

            # ---- lambda gate MLP: lam_sb[key_part, tt*16+h] ----
            lam_sb = cst.tile([128, 16 * 16], F32, tag="lam")
            with tc.tile_pool(name="pre", bufs=1) as pre:
                h1_sb = pre.tile([65, TPC], F32R, tag="h1")
                lp = pre.tile([2, TPC], F32R, tag="lp")
                nc.sync.dma_start(out=lp[:], in_=lpos32_d.ap())
                nc.sync.dma_start(out=h1_sb[64:65, :], in_=lpos32_d.ap()[1:2, :])
                for j in range(4):
                    ph = ps.tile([64, 512], F32, name=f"ph{j}", tag="b512")
                    nc.tensor.matmul(ph[:], l1_sb[:], lp[:, j * 512:(j + 1) * 512], start=True, stop=True)
                    nc.scalar.activation(h1_sb[0:64, j * 512:(j + 1) * 512], ph[:], AF.Relu)
                for tt in range(16):
                    pl = ps.tile([128, 16], F32, name=f"pl{tt}", tag="b512")
                    nc.tensor.matmul(pl[:], h1_sb[:, tt * 128:(tt + 1) * 128], l2_sb[:],
                                     start=True, stop=True)
                    nc.scalar.activation(lam_sb[:, tt * 16:(tt + 1) * 16], pl[:], AF.Sigmoid)

            # ---- main loop over chunks ----
            ctx2 = [tc.tile_pool(name="xs", bufs=12), tc.tile_pool(name="qk", bufs=16),
                    tc.tile_pool(name="vv", bufs=4), tc.tile_pool(name="pt", bufs=5),
                    tc.tile_pool(name="at", bufs=8), tc.tile_pool(name="atn", bufs=8),
                    tc.tile_pool(name="ob", bufs=3), tc.tile_pool(name="qb", bufs=3),
                    tc.tile_pool(name="xu", bufs=2)]
            xs = ctx2[0].__enter__(); qk = ctx2[1].__enter__(); vv = ctx2[2].__enter__()
            ptp = ctx2[3].__enter__(); at = ctx2[4].__enter__(); atn = ctx2[5].__enter__()
            ob = ctx2[6].__enter__(); qb = ctx2[7].__enter__(); xu = ctx2[8].__enter__()
            ALU = mybir.AluOpType
            for c in range(NCHC):
                # x tiles for this chunk: rows = features, cols = tokens.
                # Unpack 12-bit fixed point -> f16 (exact integer chain,
                # validated bit-exact on-device in isolation).
                x_sb = []
                for kt in range(8):
                    hi = xu.tile([128, 512], U8, name=f"xh_c{c}_k{kt}", tag="xh")
                    nc.sync.dma_start(
                        out=hi[:],
                        in_=xhi_d.ap()[kt * 128:(kt + 1) * 128, c * 512:(c + 1) * 512])
                    lo = xu.tile([128, 128], U8, name=f"xl_c{c}_k{kt}", tag="xl")
                    nc.sync.dma_start(
                        out=lo[:],
                        in_=xlo_d.ap()[kt * 128:(kt + 1) * 128, c * 128:(c + 1) * 128])
                    hi16 = xu.tile([128, 512], U16, name=f"xh16_c{c}_k{kt}", tag="xh16")
                    nc.vector.tensor_copy(hi16[:], hi[:])
                    hi16s = xu.tile([128, 512], U16, name=f"xh16s_c{c}_k{kt}", tag="xh16s")
                    nc.vector.tensor_scalar(hi16s[:], hi16[:], 2, None, ALU.logical_shift_left)
                    lof = xu.tile([128, 512], U8, name=f"xlf_c{c}_k{kt}", tag="xlf")
                    lof4 = lof.rearrange("p (e four) -> p e four", four=4)
                    lo3 = lo[:].rearrange("p (e o) -> p e o", o=1)
                    nc.vector.tensor_scalar(lof4[:, :, 0:1], lo3, 3, None, ALU.bitwise_and)
                    for j in range(1, 4):
                        tmp = xu.tile([128, 128], U8, name=f"xt{j}_c{c}_k{kt}", tag=f"xt{j}")
                        nc.vector.tensor_scalar(tmp[:], lo[:], 2 * j, None,
                                                ALU.logical_shift_right)
                        nc.vector.tensor_scalar(
                            lof4[:, :, j:j + 1],
                            tmp[:].rearrange("p (e o) -> p e o", o=1),
                            3, None, ALU.bitwise_and)
                    lo16 = xu.tile([128, 512], U16, name=f"xl16_c{c}_k{kt}", tag="xl16")
                    nc.vector.tensor_copy(lo16[:], lof[:])
                    q16 = xu.tile([128, 512], U16, name=f"xq16_c{c}_k{kt}", tag="xq16")
                    nc.vector.tensor_tensor(q16[:], hi16s[:], lo16[:], ALU.bitwise_or)
                    qf = xu.tile([128, 512], F32, name=f"xqf_c{c}_k{kt}", tag="xqf")
                    nc.vector.tensor_copy(qf[:], q16[:])
                    xt = xs.tile([128, 512], F16, name=f"x_c{c}_k{kt}", tag="x")
                    nc.vector.tensor_scalar(xt[:], qf[:], xdl_sb[:, 0:1], xmn_sb[:, 0:1],
                                            ALU.mult, ALU.add)
                    x_sb.append(xt)

                # q,k projection: out.T tiles [feat 128, tok 512], feats 0..2047
                qkT = []
                for ct in range(16):
                    pq = ps.tile([128, 512], F32, name=f"pq_c{c}_{ct}", tag="b512")
                    for kt in range(8):
                        nc.tensor.matmul(pq[:], wqk_sb[ct][:, kt * 128:(kt + 1) * 128],
                                         x_sb[kt][:], start=(kt == 0), stop=(kt == 7))
                    qt = qk.tile([128, 512], F16, name=f"qkT_c{c}_{ct}", tag="qkT")
                    nc.scalar.copy(qt[:], pq[:])
                    qkT.append(qt)

                # v projection (x stationary): v_sb[tt] = [tok 128, 16*(64+1)]
                v_sb = []
                for tt in range(4):
                    vt = vv.tile([128, 16 * 65], F16, name=f"v_c{c}_{tt}", tag="v")
                    v3 = vt.rearrange("p (h e) -> p h e", e=65)
                    nc.sync.dma_start(
                        out=v3[:, :, 64:65],
                        in_=logpos_d.ap()[1:2, 0:2048].rearrange("r (p e o) -> (r p) e o", p=128, o=1))
                    v_sb.append(vt)
                for g in range(2):
                    for tt in range(4):
                        pv = ps.tile([128, 512], F32, name=f"pv_c{c}_{g}_{tt}", tag="b512")
                        for kt in range(8):
                            nc.tensor.matmul(pv[:], x_sb[kt][:, tt * 128:(tt + 1) * 128],
                                             wv_sb[g * 8 + kt][:],
                                             start=(kt == 0), stop=(kt == 7))
                        dst = v_sb[tt].rearrange("p (h e) -> p h e", e=65)[:, g * 8:(g + 1) * 8, 0:64]
                        src = pv.rearrange("p (h e) -> p h e", e=64)
                        nc.scalar.copy(dst, src)

                # attention per head
                sums_sb = at.tile([16, 512], F32, name=f"sums_c{c}", tag="sums")
                attnT = []
                for hp in range(8):
                    a_t = at.tile([128, 512], F32, name=f"attnT_c{c}_{hp}", tag="attnT")
                    attnT.append(a_t)
                for h in range(16):
                    hp, r0 = h // 2, (h % 2) * 64
                    kt_tile = qkT[8 + h // 2]
                    qt_tile = qkT[h // 2]
                    p_ts = []
                    for kk in range(4):
                        pscr = ps.tile([128, 512], F32, name=f"ps_c{c}_h{h}_{kk}", tag="b512")
                        nc.tensor.matmul(pscr[:],
                                         kt_tile[r0:r0 + 64, kk * 128:(kk + 1) * 128],
                                         qt_tile[r0:r0 + 64, :], start=True, stop=True)
                        p_t = ptp.tile([128, 512], F16, name=f"p_c{c}_h{h}_{kk}", tag="p")
                        tt = c * 4 + kk
                        nc.scalar.activation(p_t[:], pscr[:], AF.Exp,
                                             scale=lam_sb[:, tt * 16 + h:tt * 16 + h + 1])
                        p_ts.append(p_t)
                    pav = ps.tile([128, 512], F32, name=f"pav_c{c}_h{h}", tag="b512")
                    for kk in range(4):
                        nc.tensor.matmul(pav[0:65, :],
                                         v_sb[kk][:, h * 65:(h + 1) * 65],
                                         p_ts[kk][:], start=(kk == 0), stop=(kk == 3))
                    nc.scalar.copy(attnT[hp][r0:r0 + 64, :], pav[0:64, :])
                    srow = at.tile([1, 512], F32, name=f"srow_c{c}_h{h}", tag="srow", bufs=2)
                    nc.scalar.copy(srow[:], pav[64:65, :])
                    nc.sync.dma_start(out=sums_sb[h:h + 1, :], in_=srow[:])

                # normalization via selector broadcast of 1/sums
                inv_sb = at.tile([16, 512], F32R, name=f"inv_c{c}", tag="inv")
                nc.vector.reciprocal(inv_sb[:], sums_sb[:])
                attnTn = []
                for hp in range(8):
                    pg = ps.tile([128, 512], F32, name=f"pg_c{c}_{hp}", tag="b512")
                    nc.tensor.matmul(pg[:], sel_sb[:, hp * 128:(hp + 1) * 128], inv_sb[:],
                                     start=True, stop=True)
                    an = atn.tile([128, 512], F16, name=f"attnTn_c{c}_{hp}", tag="an")
                    nc.vector.tensor_mul(an[:], attnT[hp][:], pg[:])
                    attnTn.append(an)

                # output projection + int8 quantization with per-row scales
                for of in range(8):
                    po = ps.tile([128, 512], F32, name=f"po_c{c}_{of}", tag="b512")
                    for kt in range(8):
                        nc.tensor.matmul(po[:], wout_sb[kt][:, of * 128:(of + 1) * 128],
                                         attnTn[kt][:], start=(kt == 0), stop=(kt == 7))
                    o_sb = ob.tile([128, 512], F32, name=f"o_c{c}_{of}", tag="o")
                    nc.scalar.activation(o_sb[:], po[:], AF.Identity,
                                         bias=bout_sb[:, of:of + 1])
                    mx = ob.tile([128, 1], F32, name=f"mx_c{c}_{of}", tag="mx", bufs=3)
                    nc.vector.tensor_reduce(mx[:], o_sb[:], mybir.AxisListType.X,
                                            mybir.AluOpType.max, apply_absolute_value=True)
                    inv127 = ob.tile([128, 1], F32, name=f"i127_c{c}_{of}", tag="i127", bufs=3)
                    nc.vector.reciprocal(inv127[:], mx[:])
                    i127s = ob.tile([128, 1], F32, name=f"i127s_c{c}_{of}", tag="i127s", bufs=3)
                    nc.scalar.activation(i127s[:], inv127[:], AF.Identity, scale=127.0)
                    nc.scalar.activation(oscale_sb[:, (of * 4 + c):(of * 4 + c) + 1],
                                         mx[:], AF.Identity, scale=1.0 / 127.0)
                    qt8 = qb.tile([128, 512], I8, name=f"q_c{c}_{of}", tag="q8")
                    nc.scalar.activation(qt8[:], o_sb[:], AF.Identity, scale=i127s[:])
                    nc.sync.dma_start(
                        out=outq_d.ap()[of * 128:(of + 1) * 128, c * 512:(c + 1) * 512],
                        in_=qt8[:])
            nc.sync.dma_start(out=oscale_d.ap(), in_=oscale_sb[:])
            for cm in reversed(ctx2):
                cm.__exit__(None, None, None)

    split_multi_waits(nc)
    return nc


_cache = {}


def _get_exec():
    """Build the program + cached jitted SPMD dispatcher (mirrors
    bass2jax.run_bass_via_pjrt minus per-call retrace and minus the
    donated zero output buffers)."""
    if "exec" in _cache:
        return _cache["exec"]
    import jax
    from jax.sharding import Mesh, PartitionSpec, NamedSharding
    from jax.experimental.shard_map import shard_map
    from concourse.bass2jax import (
        _bass_exec_p, install_neuronx_cc_hook, partition_id_tensor)

    nc = build_program()
    install_neuronx_cc_hook()

    partition_name = nc.partition_id_tensor.name if nc.partition_id_tensor else None
    in_names, out_names, out_avals = [], [], []
    for alloc in nc.m.functions[0].allocations:
        if not isinstance(alloc, mybir.MemoryLocationSet):
            continue
        name = alloc.memorylocations[0].name
        if alloc.kind == "ExternalInput":
            if name != partition_name:
                in_names.append(name)
        elif alloc.kind == "ExternalOutput":
            out_names.append(name)
            out_avals.append(jax.core.ShapedArray(
                tuple(alloc.tensor_shape), mybir.dt.np(alloc.dtype)))
    n_params = len(in_names)
    n_outs = len(out_avals)
    # NEFF output tensors are custom-call *results* (out_rename overrides
    # the input binding in neuronx_cc_hook), and this kernel writes every
    # output element — so no donated zero buffers are needed at all.
    in_names_full = in_names + ([partition_name] if partition_name else [])

    def _body(*args):
        operands = list(args)
        if partition_name is not None:
            operands.append(partition_id_tensor())
        return tuple(_bass_exec_p.bind(
            *operands, out_avals=tuple(out_avals), in_names=tuple(in_names_full),
            out_names=tuple(out_names), lowering_input_output_aliases=(),
            sim_require_finite=True, sim_require_nnan=True, nc=nc))

    devices = jax.devices()[:NCORES]
    mesh = Mesh(np.asarray(devices), ("core",))
    sh = NamedSharding(mesh, PartitionSpec("core"))
    in_specs = (PartitionSpec("core"),) * n_params
    out_specs = (PartitionSpec("core"),) * n_outs
    sharded = jax.jit(
        shard_map(_body, mesh=mesh, in_specs=in_specs, out_specs=out_specs,
                  check_rep=False),
        keep_unused=True)

    # Input-independent constants: upload once, reuse every dispatch.
    logpos32 = np.empty((NCORES * 2, TPC), np.float32)
    for core in range(NCORES):
        pos = (core % 2) * TPC + np.arange(TPC, dtype=np.float32)
        logpos32[core * 2] = np.log(pos + 1.0)
        logpos32[core * 2 + 1] = 1.0
    sel = (np.arange(C)[None, :] // DH == np.arange(H)[:, None]).astype(np.float32)
    sel_all = np.broadcast_to(sel, (NCORES, H, C)).reshape(NCORES * H, C)
    consts = {
        "logpos16": jax.device_put(logpos32.astype(np.float16), sh),
        "logpos32": jax.device_put(logpos32, sh),
        "sel": jax.device_put(np.ascontiguousarray(sel_all), sh),
    }
    jax.block_until_ready(list(consts.values()))

    ex = dict(nc=nc, sharded=sharded, in_names=in_names, out_names=out_names,
              consts=consts, devices=devices, sh=sh)
    _cache["exec"] = ex
    return ex


def dispatch(in_maps):
    """The timed region: host numpy in_maps -> per-core fp32 outT arrays.
    Uploads fp16 x + sharded fp16 weights, executes (on-device weight
    AllGather + attention), downloads int8 output + scales (per-shard in
    parallel threads — the tunnel fetch path is ~1.6x faster that way),
    dequantizes per core as each shard arrives."""
    from concurrent.futures import ThreadPoolExecutor
    import jax
    ex = _get_exec()
    consts = ex["consts"]
    sh = ex["sh"]
    devices = ex["devices"]
    args = []
    for name in ex["in_names"]:
        if name in consts:
            args.append(consts[name])
            continue
        # upload per-core shards directly (skips a host-side concat copy)
        shards = jax.device_put([m[name] for m in in_maps], devices)
        per = in_maps[0][name].shape[0]
        glob = jax.make_array_from_single_device_arrays(
            (NCORES * per, *in_maps[0][name].shape[1:]), sh, shards)
        args.append(glob)
    out_arrs = ex["sharded"](*args)
    q_shards = sorted(out_arrs[0].addressable_shards, key=lambda s: s.index[0].start)
    s_shards = sorted(out_arrs[1].addressable_shards, key=lambda s: s.index[0].start)

    with ThreadPoolExecutor(NCORES + 2) as pool:
        # scales are tiny; fetch them concurrently with the big int8 shards
        s_futs = [pool.submit(lambda shd: np.asarray(shd.data), shd) for shd in s_shards]

        def fetch_decode(core):
            q = np.asarray(q_shards[core].data).reshape(8, 128, 4, 512)
            s = s_futs[core].result().reshape(128, 8, 4).transpose(1, 0, 2)
            o = np.empty((8, 128, 4, 512), np.float32)
            np.multiply(q, s[..., None], out=o)
            return o.reshape(C, TPC)

        outs = list(pool.map(fetch_decode, range(NCORES)))
    return outs


def kernel(x, Wqkv, Wout, bout, Wl1, bl1, Wl2, bl2):
    scale = DH ** -0.5
    wqkvT = np.ascontiguousarray(Wqkv.T, dtype=np.float32)
    wqkvT[:, :C] *= scale  # fold attention scale into q projection (1/8: exact in fp16)
    woutT = Wout.T.astype(np.float32)
    lamw32 = np.empty((1, 2448), np.float32)
    lamw32[0, 0:128] = np.stack([Wl1[:, 0], bl1]).astype(np.float32).ravel()
    lamw32[0, 128:1168] = np.concatenate(
        [np.asarray(Wl2).T, np.asarray(bl2)[None, :]], 0).astype(np.float32).ravel()
    lamw32[0, 1168:2192] = np.asarray(bout).reshape(8, 128).T.astype(np.float32).ravel()

    xf = np.asarray(x, dtype=np.float32)
    wqkvT16 = wqkvT.astype(np.float16)
    woutT16 = woutT.astype(np.float16)

    in_maps = []
    for core in range(NCORES):
        b, half = core // 2, core % 2
        xT = xf[b, half * TPC:(half + 1) * TPC, :].T
        xmin = float(xT.min())
        delta = max((float(xT.max()) - xmin) / 1023.0, 1e-30)
        q = np.clip(np.round((xT - xmin) * (1.0 / delta)), 0, 1023).astype(np.uint16)
        xhi = (q >> 2).astype(np.uint8)
        lo2 = (q & 3).astype(np.uint8)
        xlo = np.ascontiguousarray(
            lo2[:, 0::4] | (lo2[:, 1::4] << 2) | (lo2[:, 2::4] << 4) | (lo2[:, 3::4] << 6))
        w16 = np.empty((C, WPK), np.float16)
        w16[:, :WQC] = wqkvT16[:, core * WQC:(core + 1) * WQC]
        w16[:, WQC:] = woutT16[:, core * WOC:(core + 1) * WOC]
        lamw = lamw32.copy()
        lamw[0, 2192:2320] = delta
        lamw[0, 2320:2448] = xmin
        in_maps.append(dict(xhi=np.ascontiguousarray(xhi), xlo=xlo,
                            w16=w16, lamw32=lamw))

    global _last_in_maps
    _last_in_maps = in_maps
    res = dispatch(in_maps)

    out = np.empty((B, T, C), np.float32)
    for core in range(NCORES):
        b, half = core // 2, core % 2
        out[b, half * TPC:(half + 1) * TPC, :] = res[core].T
    return out
